# revision 14
# baseline (speedup 1.0000x reference)
"""Bass/Trainium2 kernel for nn_BilinearInteraction.

Computes out[b, p, :] = (x[b, i_p, :] @ W[p].T + bias[p]) * x[b, j_p, :]
for the 325 upper-triangular field pairs (i_p < j_p), batch B=4096,
F=26 fields, D=32 embed dim.

Strategy (data parallel over 8 NeuronCores, 512 batch rows each):
  - batch rows live on SBUF partitions (4 tiles of 128 rows per core).
  - pairs sharing the same i-field are contiguous in p, so for each field
    f the output columns [pstart(f)*32, (pstart(f)+25-f)*32) are produced
    by ONE stationary operand: xT_f = transpose(x[:, f, :]) augmented
    with a row of ones (bias trick -> K=33 contraction).
  - fields are stacked 3-at-a-time (block-diagonal, K=99) so one PE
    transpose + one psum->sbuf copy serves 3 fields. The block-diagonal
    weight image lives only in SBUF: HBM holds the compact wbs[33,10400]
    (rows 0..31 = W[p].T per pair, row 32 = bias); 25 small DMAs scatter
    it into the zeroed [99,10400] SBUF tile at phase offsets 33*(f%3).
  - matmul inputs are float32r (single-pass fp32 on the PE, ~2 cyc/col).
  - psum * v_j (v_j = natural column slices of x) runs on VectorE, with
    the first two field-groups offloaded to GpSimd (via a ScalarE
    psum->sbuf copy) to keep VectorE off the critical path.
  - results land in four [128, ~2600] quarter tiles; each quarter DMAs
    to HBM as soon as its last column is written (the 21.3 MB/core
    output write is the roofline term).
"""

import os
import sys

import numpy as np

for _p in (
    "/root/.axon_site",
    "/root/.axon_site/_ro/trn_rl_repo",
    "/root/.axon_site/_ro/pypackages",
    "/opt/trn_rl_repo",
):
    if os.path.isdir(_p) and _p not in sys.path:
        sys.path.append(_p)

import concourse.bacc as bacc
import concourse.tile as tile
from concourse import mybir
from concourse.bass_utils import run_bass_kernel_spmd
from concourse.masks import make_identity

N_CORES = 8
B, F, D = 4096, 26, 32
NPAIR = 325  # F*(F-1)/2
BLOC = B // N_CORES  # 512 batch rows per core
PB = 128  # batch rows per tile (partition dim)
NT = BLOC // PB  # 4 tiles per core
OUTW = NPAIR * D  # 10400 output columns
DA = D + 1  # field block width in augmented x (32 data + 1 one)
NQ = 4  # output quarter tiles per batch tile
# quarter boundaries, 32-aligned (pair-aligned): widths 2624/2592/2592/2592
Q_BOUNDS = [0, 2624, 5216, 7808, OUTW]

FP32 = mybir.dt.float32
FP32R = mybir.dt.float32r
BF16 = mybir.dt.bfloat16

# matmul input dtype: "f32r" (~2 cyc/col, ~1e-3 elemwise err),
# "bf16" (1 cyc/col, ~1e-2), "f32" (exact, 4 cyc/col)
MM_MODE = os.environ.get("BILIN_MM_MODE", "f32r")
_MM_DT = {"bf16": BF16, "f32r": FP32R, "f32": FP32}[MM_MODE]

# 3-field stacks for block-diagonal matmuls (field 24 alone)
GROUPS = [tuple(range(g, min(g + 3, 25))) for g in range(0, 25, 3)]
# field-aligned wbs column chunks (zero-fill + scatter pipelining)
WB_CHUNKS = [(0, 3), (3, 7), (7, 13), (13, 25)]
# (group, chunk_idx) pairs whose psum*v_j multiply runs on GpSimd
# (mid-kernel groups: keeps VectorE free for the first output quarter and
# GpSimd free for startup zero-fills)
OFFLOAD = {(2, 0), (2, 1), (2, 2), (2, 3), (3, 0), (3, 1)}


def _pstart(f: int) -> int:
    # first pair index whose i == f (pairs sorted by (i, j))
    return 25 * f - f * (f - 1) // 2


def _nf(f: int) -> int:
    return (25 - f) * D  # output columns owned by field f


def _chunks(width: int):
    """Split width into psum chunks, each <=512, >=256 where possible, %32==0."""
    out = []
    rem = width
    while rem > 0:
        if rem <= 512:
            c = rem
        elif rem >= 768:
            c = 512
        else:
            c = rem - 256
        out.append(c)
        rem -= c
    starts = []
    s = 0
    for c in out:
        starts.append((s, s + c))
        s += c
    return starts


def pack_weights(W: np.ndarray, b: np.ndarray) -> np.ndarray:
    """Build compact wbs[33, OUTW]: rows 0..31 = W[p].T per pair, row 32 = bias."""
    W = np.ascontiguousarray(W, dtype=np.float32)
    b = np.ascontiguousarray(b, dtype=np.float32)
    wbs = np.empty((33, OUTW), dtype=np.float32)
    wbs[:D] = W.transpose(2, 0, 1).reshape(D, OUTW)
    wbs[D] = b.reshape(OUTW)
    if MM_MODE == "bf16":
        import ml_dtypes

        wbs = wbs.astype(ml_dtypes.bfloat16)
    return wbs


def _emit(tc: tile.TileContext, out_ap, xs_ap, wbs_ap):
    from contextlib import ExitStack

    nc = tc.nc
    mm_dt = _MM_DT

    with ExitStack() as ctx:
        const = ctx.enter_context(tc.tile_pool(name="const", bufs=1))
        xp = ctx.enter_context(tc.tile_pool(name="xp", bufs=4))
        stg = ctx.enter_context(tc.tile_pool(name="stg", bufs=2))
        op = ctx.enter_context(tc.tile_pool(name="op", bufs=2))
        scr = ctx.enter_context(tc.tile_pool(name="scr", bufs=4))
        tps = ctx.enter_context(tc.tile_pool(name="tps", bufs=2, space="PSUM"))
        mps = ctx.enter_context(tc.tile_pool(name="mps", bufs=6, space="PSUM"))

        if mm_dt == FP32R:
            # gpsimd memset can't write f32r; build fp32 then round via ACT
            ident_nat = const.tile([128, 128], FP32, tag="idnat", name="ident_nat")
            make_identity(nc, ident_nat)
            ident = const.tile([128, 128], mm_dt, tag="ident", name="ident")
            nc.scalar.copy(ident, ident_nat)
            ones_nat = const.tile([128, F], FP32, tag="ones", name="ones_nat")
            nc.gpsimd.memset(ones_nat, 1.0)
        else:
            ident = const.tile([128, 128], mm_dt, tag="ident", name="ident")
            make_identity(nc, ident)
            ones_nat = None

        # x loads go first so they are not queued behind the weight scatters
        def load_x(bt):
            b0 = bt * PB
            # x loaded contiguously (clean 3.3KB/partition DMA runs)
            x_nat = xp.tile([PB, F, D], FP32, tag="xnat", name=f"xnat{bt}")
            nc.sync.dma_start(out=x_nat, in_=xs_ap[b0 : b0 + PB])
            return x_nat

        x_nats = [load_x(bt) for bt in range(NT)]

        # block-diagonal weight image: zero-fill, then scatter the compact
        # HBM wbs into phase rows 33*(f%3) per field, chunk by chunk.
        # (gpsimd/DVE memsets can't write f32r -> zero an fp32 scratch and
        # round it in via ACT copy, the same producer class as staging.
        # Scatter DMAs issue from the ACT queue so they naturally follow
        # the zero-copies without clogging the sync queue.)
        wbs3 = const.tile([99, OUTW], mm_dt, tag="wbs3", name="wbs3")
        zeng = [nc.gpsimd, nc.vector, nc.gpsimd, nc.vector]
        for i, (f0, f1) in enumerate(WB_CHUNKS):
            c0 = _pstart(f0) * D
            c1 = (_pstart(f1) if f1 < 25 else NPAIR) * D
            if mm_dt == FP32R:
                zc = scr.tile(
                    [99, c1 - c0], FP32, tag=f"zc{i % 2}", bufs=1, name=f"zc{f0}"
                )
                zeng[i].memset(zc, 0.0)
                nc.scalar.copy(wbs3[:, c0:c1], zc)
            else:
                zeng[i].memset(wbs3[:, c0:c1], 0.0)
        for f in range(25):
            ph = f % 3
            base = _pstart(f) * D
            nc.scalar.dma_start(
                out=wbs3[33 * ph : 33 * ph + 33, base : base + _nf(f)],
                in_=wbs_ap[:, base : base + _nf(f)],
            )

        def prep_x(bt, x_nat):
            # cast copy into mm dtype with a ones column per field block
            x_mm = xp.tile([PB, F, DA], mm_dt, tag="xmm", name=f"xmm{bt}")
            nc.scalar.copy(x_mm[:, :, 0:D], x_nat)
            if mm_dt == FP32R:
                nc.scalar.copy(x_mm[:, :, D], ones_nat)
            else:
                nc.gpsimd.memset(x_mm[:, :, D : D + 1], 1.0)
            return x_mm

        def transposes(bt, x_mm):
            # transpose each field stack -> [33*len(fs), 128] staging (PE+ACT)
            stgs = []
            for gi, fs in enumerate(GROUPS):
                kg = 33 * len(fs)
                tin = x_mm[:, fs[0] : fs[0] + len(fs), :].rearrange("p a b -> p (a b)")
                ps_t = tps.tile([kg, PB], mm_dt, tag="tp", name=f"pst{bt}_{gi}")
                nc.tensor.transpose(ps_t, tin, ident)
                stg_g = stg.tile([kg, PB], mm_dt, tag=f"sg{gi}", name=f"stg{bt}_{gi}")
                nc.scalar.copy(stg_g, ps_t)
                stgs.append(stg_g)
            return stgs

        x_mms = {0: prep_x(0, x_nats[0])}
        stgs_by_bt = {0: transposes(0, x_mms[0])}

        for bt in range(NT):
            b0 = bt * PB
            x_nat = x_nats[bt]
            stgs = stgs_by_bt.pop(bt)

            quarters = [
                op.tile(
                    [PB, Q_BOUNDS[q + 1] - Q_BOUNDS[q]],
                    FP32,
                    tag=f"osb{q}",
                    name=f"osb{bt}_{q}",
                )
                for q in range(NQ)
            ]

            def tt(col0, col1, src, scol0, j0, eng):
                """quarters[col0:col1) = src * v_j, split at quarter edges."""
                c = col0
                while c < col1:
                    q = next(i for i in range(NQ) if c < Q_BOUNDS[i + 1])
                    e = min(col1, Q_BOUNDS[q + 1])
                    nj = (e - c) // D
                    j = j0 + (c - col0) // D
                    eng.tensor_mul(
                        quarters[q][:, c - Q_BOUNDS[q] : e - Q_BOUNDS[q]],
                        src[:, scol0 + (c - col0) : scol0 + (e - col0)],
                        x_nat[:, j : j + nj, :],
                    )
                    c = e

            for gi, fs in enumerate(GROUPS):
                if gi == 5 and bt + 1 < NT:
                    # interleave next tile's transposes to keep the PE dense
                    x_mms[bt + 1] = prep_x(bt + 1, x_nats[bt + 1])
                    stgs_by_bt[bt + 1] = transposes(bt + 1, x_mms[bt + 1])
                kg = 33 * len(fs)
                gbase = _pstart(fs[0]) * D
                width = sum(_nf(f) for f in fs)
                offs = []
                o = 0
                for f in fs:
                    offs.append(o)
                    o += _nf(f)
                for ci, (c0, c1) in enumerate(_chunks(width)):
                    offload = (gi, ci) in OFFLOAD
                    ps_m = mps.tile(
                        [PB, c1 - c0], FP32, tag="mp", name=f"psm{bt}_{gi}_{c0}"
                    )
                    nc.tensor.matmul(
                        ps_m,
                        stgs[gi],
                        wbs3[:kg, gbase + c0 : gbase + c1],
                        start=True,
                        stop=True,
                    )
                    if offload:
                        # GpSimd cannot read PSUM: stage via ScalarE copy
                        sc = scr.tile(
                            [PB, c1 - c0], FP32, tag="sc", name=f"sc{bt}_{gi}_{c0}"
                        )
                        nc.scalar.copy(sc, ps_m)
                        src = sc
                    else:
                        src = ps_m
                    eng = nc.gpsimd if offload else nc.vector
                    for f, off in zip(fs, offs):
                        s0 = max(c0, off)
                        s1 = min(c1, off + _nf(f))
                        if s0 >= s1:
                            continue
                        tt(
                            gbase + s0,
                            gbase + s1,
                            src,
                            s0 - c0,
                            f + 1 + (s0 - off) // D,
                            eng,
                        )

            for q in range(NQ):
                nc.sync.dma_start(
                    out=out_ap[b0 : b0 + PB, Q_BOUNDS[q] : Q_BOUNDS[q + 1]],
                    in_=quarters[q],
                )


_CACHE = {}


def _build():
    if "nc" in _CACHE:
        return _CACHE["nc"]
    nc = bacc.Bacc("TRN2", target_bir_lowering=False, debug=False)
    xs = nc.dram_tensor("xs", [BLOC, F, D], FP32, kind="ExternalInput").ap()
    wbs = nc.dram_tensor("wbs", [33, OUTW], _MM_DT, kind="ExternalInput").ap()
    out = nc.dram_tensor("out", [BLOC, OUTW], FP32, kind="ExternalOutput").ap()
    with tile.TileContext(nc) as tc:
        _emit(tc, out, xs, wbs)
    nc.compile()
    _CACHE["nc"] = nc
    return nc


def run(
    x: np.ndarray,
    W: np.ndarray,
    b: np.ndarray,
    trace: bool = False,
    tmpdir: str | None = None,
):
    """Shard, execute on 8 cores, gather. Returns (out, results_obj)."""
    x = np.ascontiguousarray(x, dtype=np.float32)
    wbs = pack_weights(W, b)
    nc = _build()
    in_maps = [
        {"xs": x[c * BLOC : (c + 1) * BLOC], "wbs": wbs} for c in range(N_CORES)
    ]
    res = run_bass_kernel_spmd(
        nc, in_maps, core_ids=list(range(N_CORES)), trace=trace, tmpdir=tmpdir
    )
    parts = [res.results[c]["out"].reshape(BLOC, NPAIR, D) for c in range(N_CORES)]
    out = np.concatenate(parts, axis=0).astype(np.float32, copy=False)
    return out, res


def kernel(x: np.ndarray, W: np.ndarray, b: np.ndarray) -> np.ndarray:
    out, _ = run(x, W, b, trace=False)
    return out


if __name__ == "__main__":
    rng = np.random.default_rng(0)
    x = rng.standard_normal((B, F, D), dtype=np.float32)
    W = rng.standard_normal((NPAIR, D, D), dtype=np.float32) / np.sqrt(D)
    b = rng.standard_normal((NPAIR, D), dtype=np.float32) * 0.01
    out = kernel(x, W, b)
    print("out", out.shape, out.dtype)


# revision 15
# speedup vs baseline: 1.1872x; 1.1872x over previous
"""Bass/Trainium2 kernel for nn_BilinearInteraction.

Computes out[b, p, :] = (x[b, i_p, :] @ W[p].T + bias[p]) * x[b, j_p, :]
for the 325 upper-triangular field pairs (i_p < j_p), batch B=4096,
F=26 fields, D=32 embed dim.

Strategy (data parallel over 8 NeuronCores, 512 batch rows each):
  - batch rows live on SBUF partitions (4 tiles of 128 rows per core).
  - pairs sharing the same i-field are contiguous in p, so for each field
    f the output columns [pstart(f)*32, (pstart(f)+25-f)*32) are produced
    by ONE stationary operand: xT_f = transpose(x[:, f, :]) augmented
    with a row of ones (bias trick -> K=33 contraction).
  - fields are stacked 3-at-a-time (block-diagonal, K=99) so one PE
    transpose + one psum->sbuf copy serves 3 fields. The block-diagonal
    weight image lives only in SBUF: HBM holds the compact wbs[33,10400]
    (rows 0..31 = W[p].T per pair, row 32 = bias); 25 small DMAs scatter
    it into the zeroed [99,10400] SBUF tile at phase offsets 33*(f%3).
  - matmul inputs are float32r (single-pass fp32 on the PE, ~2 cyc/col).
  - psum * v_j (v_j = natural column slices of x) runs on VectorE, with
    the first two field-groups offloaded to GpSimd (via a ScalarE
    psum->sbuf copy) to keep VectorE off the critical path.
  - results land in four [128, ~2600] quarter tiles; each quarter DMAs
    to HBM as soon as its last column is written (the 21.3 MB/core
    output write is the roofline term).
"""

import os
import sys

import numpy as np

for _p in (
    "/root/.axon_site",
    "/root/.axon_site/_ro/trn_rl_repo",
    "/root/.axon_site/_ro/pypackages",
    "/opt/trn_rl_repo",
):
    if os.path.isdir(_p) and _p not in sys.path:
        sys.path.append(_p)

import concourse.bacc as bacc
import concourse.tile as tile
from concourse import mybir
from concourse.bass_utils import run_bass_kernel_spmd
from concourse.masks import make_identity

N_CORES = 8
B, F, D = 4096, 26, 32
NPAIR = 325  # F*(F-1)/2
BLOC = B // N_CORES  # 512 batch rows per core
PB = 128  # batch rows per tile (partition dim)
NT = BLOC // PB  # 4 tiles per core
OUTW = NPAIR * D  # 10400 output columns
DA = D + 1  # field block width in augmented x (32 data + 1 one)
NQ = 4  # output quarter tiles per batch tile
# quarter boundaries, 32-aligned (pair-aligned): widths 2624/2592/2592/2592
Q_BOUNDS = [0, 2624, 5216, 7808, OUTW]

FP32 = mybir.dt.float32
FP32R = mybir.dt.float32r
BF16 = mybir.dt.bfloat16

# matmul input dtype: "f32r" (~2 cyc/col, ~1e-3 elemwise err),
# "bf16" (1 cyc/col, ~1e-2), "f32" (exact, 4 cyc/col)
MM_MODE = os.environ.get("BILIN_MM_MODE", "f32r")
_MM_DT = {"bf16": BF16, "f32r": FP32R, "f32": FP32}[MM_MODE]

# 3-field stacks for block-diagonal matmuls (field 24 alone)
GROUPS = [tuple(range(g, min(g + 3, 25))) for g in range(0, 25, 3)]
# field-aligned wbs column chunks (zero-fill + scatter pipelining)
WB_CHUNKS = [(0, 3), (3, 7), (7, 13), (13, 25)]
# (group, chunk_idx) pairs whose psum*v_j multiply runs on GpSimd
OFFLOAD = {(0, 0), (0, 1), (0, 2), (0, 3), (0, 4), (1, 0), (1, 1)}


def _pstart(f: int) -> int:
    # first pair index whose i == f (pairs sorted by (i, j))
    return 25 * f - f * (f - 1) // 2


def _nf(f: int) -> int:
    return (25 - f) * D  # output columns owned by field f


def _chunks(width: int):
    """Split width into psum chunks, each <=512, >=256 where possible, %32==0."""
    out = []
    rem = width
    while rem > 0:
        if rem <= 512:
            c = rem
        elif rem >= 768:
            c = 512
        else:
            c = rem - 256
        out.append(c)
        rem -= c
    starts = []
    s = 0
    for c in out:
        starts.append((s, s + c))
        s += c
    return starts


def pack_weights(W: np.ndarray, b: np.ndarray) -> np.ndarray:
    """Build compact wbs[33, OUTW]: rows 0..31 = W[p].T per pair, row 32 = bias."""
    W = np.ascontiguousarray(W, dtype=np.float32)
    b = np.ascontiguousarray(b, dtype=np.float32)
    wbs = np.empty((33, OUTW), dtype=np.float32)
    wbs[:D] = W.transpose(2, 0, 1).reshape(D, OUTW)
    wbs[D] = b.reshape(OUTW)
    if MM_MODE == "bf16":
        import ml_dtypes

        wbs = wbs.astype(ml_dtypes.bfloat16)
    return wbs


def _emit(tc: tile.TileContext, out_ap, xs_ap, wbs_ap):
    from contextlib import ExitStack

    nc = tc.nc
    mm_dt = _MM_DT

    with ExitStack() as ctx:
        const = ctx.enter_context(tc.tile_pool(name="const", bufs=1))
        xp = ctx.enter_context(tc.tile_pool(name="xp", bufs=4))
        stg = ctx.enter_context(tc.tile_pool(name="stg", bufs=2))
        op = ctx.enter_context(tc.tile_pool(name="op", bufs=2))
        scr = ctx.enter_context(tc.tile_pool(name="scr", bufs=4))
        tps = ctx.enter_context(tc.tile_pool(name="tps", bufs=2, space="PSUM"))
        mps = ctx.enter_context(tc.tile_pool(name="mps", bufs=6, space="PSUM"))

        if mm_dt == FP32R:
            # gpsimd memset can't write f32r; build fp32 then round via ACT
            ident_nat = const.tile([128, 128], FP32, tag="idnat", name="ident_nat")
            make_identity(nc, ident_nat)
            ident = const.tile([128, 128], mm_dt, tag="ident", name="ident")
            nc.scalar.copy(ident, ident_nat)
            ones_nat = const.tile([128, F], FP32, tag="ones", name="ones_nat")
            nc.gpsimd.memset(ones_nat, 1.0)
        else:
            ident = const.tile([128, 128], mm_dt, tag="ident", name="ident")
            make_identity(nc, ident)
            ones_nat = None

        # x loads go first so they are not queued behind the weight scatters
        def load_x(bt):
            b0 = bt * PB
            # x loaded contiguously (clean 3.3KB/partition DMA runs)
            x_nat = xp.tile([PB, F, D], FP32, tag="xnat", name=f"xnat{bt}")
            nc.sync.dma_start(out=x_nat, in_=xs_ap[b0 : b0 + PB])
            return x_nat

        x_nats = [load_x(bt) for bt in range(NT)]

        # block-diagonal weight image: zero-fill, then scatter the compact
        # HBM wbs into phase rows 33*(f%3) per field, chunk by chunk.
        # (gpsimd/DVE memsets can't write f32r -> zero an fp32 scratch and
        # round it in via ACT copy, the same producer class as staging.
        # Scatter DMAs issue from the ACT queue so they naturally follow
        # the zero-copies without clogging the sync queue.)
        wbs3 = const.tile([99, OUTW], mm_dt, tag="wbs3", name="wbs3")
        zeng = [nc.gpsimd, nc.vector, nc.gpsimd, nc.vector]
        for i, (f0, f1) in enumerate(WB_CHUNKS):
            c0 = _pstart(f0) * D
            c1 = (_pstart(f1) if f1 < 25 else NPAIR) * D
            if mm_dt == FP32R:
                zc = scr.tile(
                    [99, c1 - c0], FP32, tag=f"zc{i % 2}", bufs=1, name=f"zc{f0}"
                )
                zeng[i].memset(zc, 0.0)
                nc.scalar.copy(wbs3[:, c0:c1], zc)
            else:
                zeng[i].memset(wbs3[:, c0:c1], 0.0)
        for f in range(25):
            ph = f % 3
            base = _pstart(f) * D
            nc.sync.dma_start(
                out=wbs3[33 * ph : 33 * ph + 33, base : base + _nf(f)],
                in_=wbs_ap[:, base : base + _nf(f)],
            )

        def prep_x(bt, x_nat):
            # cast copy into mm dtype with a ones column per field block
            x_mm = xp.tile([PB, F, DA], mm_dt, tag="xmm", name=f"xmm{bt}")
            nc.scalar.copy(x_mm[:, :, 0:D], x_nat)
            if mm_dt == FP32R:
                nc.scalar.copy(x_mm[:, :, D], ones_nat)
            else:
                nc.gpsimd.memset(x_mm[:, :, D : D + 1], 1.0)
            return x_mm

        def transposes(bt, x_mm):
            # transpose each field stack -> [33*len(fs), 128] staging (PE+ACT)
            stgs = []
            for gi, fs in enumerate(GROUPS):
                kg = 33 * len(fs)
                tin = x_mm[:, fs[0] : fs[0] + len(fs), :].rearrange("p a b -> p (a b)")
                ps_t = tps.tile([kg, PB], mm_dt, tag="tp", name=f"pst{bt}_{gi}")
                nc.tensor.transpose(ps_t, tin, ident)
                stg_g = stg.tile([kg, PB], mm_dt, tag=f"sg{gi}", name=f"stg{bt}_{gi}")
                nc.scalar.copy(stg_g, ps_t)
                stgs.append(stg_g)
            return stgs

        x_mms = {0: prep_x(0, x_nats[0])}
        stgs_by_bt = {0: transposes(0, x_mms[0])}

        for bt in range(NT):
            b0 = bt * PB
            x_nat = x_nats[bt]
            stgs = stgs_by_bt.pop(bt)

            quarters = [
                op.tile(
                    [PB, Q_BOUNDS[q + 1] - Q_BOUNDS[q]],
                    FP32,
                    tag=f"osb{q}",
                    name=f"osb{bt}_{q}",
                )
                for q in range(NQ)
            ]

            def tt(col0, col1, src, scol0, j0, eng):
                """quarters[col0:col1) = src * v_j, split at quarter edges."""
                c = col0
                while c < col1:
                    q = next(i for i in range(NQ) if c < Q_BOUNDS[i + 1])
                    e = min(col1, Q_BOUNDS[q + 1])
                    nj = (e - c) // D
                    j = j0 + (c - col0) // D
                    eng.tensor_mul(
                        quarters[q][:, c - Q_BOUNDS[q] : e - Q_BOUNDS[q]],
                        src[:, scol0 + (c - col0) : scol0 + (e - col0)],
                        x_nat[:, j : j + nj, :],
                    )
                    c = e

            for gi, fs in enumerate(GROUPS):
                if gi == 5 and bt + 1 < NT:
                    # interleave next tile's transposes to keep the PE dense
                    x_mms[bt + 1] = prep_x(bt + 1, x_nats[bt + 1])
                    stgs_by_bt[bt + 1] = transposes(bt + 1, x_mms[bt + 1])
                kg = 33 * len(fs)
                gbase = _pstart(fs[0]) * D
                width = sum(_nf(f) for f in fs)
                offs = []
                o = 0
                for f in fs:
                    offs.append(o)
                    o += _nf(f)
                for ci, (c0, c1) in enumerate(_chunks(width)):
                    offload = (gi, ci) in OFFLOAD
                    ps_m = mps.tile(
                        [PB, c1 - c0], FP32, tag="mp", name=f"psm{bt}_{gi}_{c0}"
                    )
                    nc.tensor.matmul(
                        ps_m,
                        stgs[gi],
                        wbs3[:kg, gbase + c0 : gbase + c1],
                        start=True,
                        stop=True,
                    )
                    if offload:
                        # GpSimd cannot read PSUM: stage via ScalarE copy
                        sc = scr.tile(
                            [PB, c1 - c0], FP32, tag="sc", name=f"sc{bt}_{gi}_{c0}"
                        )
                        nc.scalar.copy(sc, ps_m)
                        src = sc
                    else:
                        src = ps_m
                    eng = nc.gpsimd if offload else nc.vector
                    for f, off in zip(fs, offs):
                        s0 = max(c0, off)
                        s1 = min(c1, off + _nf(f))
                        if s0 >= s1:
                            continue
                        tt(
                            gbase + s0,
                            gbase + s1,
                            src,
                            s0 - c0,
                            f + 1 + (s0 - off) // D,
                            eng,
                        )

            for q in range(NQ):
                nc.sync.dma_start(
                    out=out_ap[b0 : b0 + PB, Q_BOUNDS[q] : Q_BOUNDS[q + 1]],
                    in_=quarters[q],
                )


_CACHE = {}


def _build():
    if "nc" in _CACHE:
        return _CACHE["nc"]
    nc = bacc.Bacc("TRN2", target_bir_lowering=False, debug=False)
    xs = nc.dram_tensor("xs", [BLOC, F, D], FP32, kind="ExternalInput").ap()
    wbs = nc.dram_tensor("wbs", [33, OUTW], _MM_DT, kind="ExternalInput").ap()
    out = nc.dram_tensor("out", [BLOC, OUTW], FP32, kind="ExternalOutput").ap()
    with tile.TileContext(nc) as tc:
        _emit(tc, out, xs, wbs)
    nc.compile()
    _CACHE["nc"] = nc
    return nc


def run(
    x: np.ndarray,
    W: np.ndarray,
    b: np.ndarray,
    trace: bool = False,
    tmpdir: str | None = None,
):
    """Shard, execute on 8 cores, gather. Returns (out, results_obj)."""
    x = np.ascontiguousarray(x, dtype=np.float32)
    wbs = pack_weights(W, b)
    nc = _build()
    in_maps = [
        {"xs": x[c * BLOC : (c + 1) * BLOC], "wbs": wbs} for c in range(N_CORES)
    ]
    res = run_bass_kernel_spmd(
        nc, in_maps, core_ids=list(range(N_CORES)), trace=trace, tmpdir=tmpdir
    )
    parts = [res.results[c]["out"].reshape(BLOC, NPAIR, D) for c in range(N_CORES)]
    out = np.concatenate(parts, axis=0).astype(np.float32, copy=False)
    return out, res


def kernel(x: np.ndarray, W: np.ndarray, b: np.ndarray) -> np.ndarray:
    out, _ = run(x, W, b, trace=False)
    return out


if __name__ == "__main__":
    rng = np.random.default_rng(0)
    x = rng.standard_normal((B, F, D), dtype=np.float32)
    W = rng.standard_normal((NPAIR, D, D), dtype=np.float32) / np.sqrt(D)
    b = rng.standard_normal((NPAIR, D), dtype=np.float32) * 0.01
    out = kernel(x, W, b)
    print("out", out.shape, out.dtype)


# revision 16
# speedup vs baseline: 1.1968x; 1.0081x over previous
"""Bass/Trainium2 kernel for nn_BilinearInteraction.

Computes out[b, p, :] = (x[b, i_p, :] @ W[p].T + bias[p]) * x[b, j_p, :]
for the 325 upper-triangular field pairs (i_p < j_p), batch B=4096,
F=26 fields, D=32 embed dim.

Strategy (data parallel over 8 NeuronCores, 512 batch rows each):
  - batch rows live on SBUF partitions (4 tiles of 128 rows per core).
  - pairs sharing the same i-field are contiguous in p, so for each field
    f the output columns [pstart(f)*32, (pstart(f)+25-f)*32) are produced
    by ONE stationary operand: xT_f = transpose(x[:, f, :]) augmented
    with a row of ones (bias trick -> K=33 contraction).
  - fields are stacked 3-at-a-time (block-diagonal, K=99) so one PE
    transpose + one psum->sbuf copy serves 3 fields. The block-diagonal
    weight image lives only in SBUF: HBM holds the compact wbs[33,10400]
    (rows 0..31 = W[p].T per pair, row 32 = bias); 25 small DMAs scatter
    it into the zeroed [99,10400] SBUF tile at phase offsets 33*(f%3).
  - matmul inputs are float32r (single-pass fp32 on the PE, ~2 cyc/col).
  - psum * v_j (v_j = natural column slices of x) runs on VectorE, with
    the first two field-groups offloaded to GpSimd (via a ScalarE
    psum->sbuf copy) to keep VectorE off the critical path.
  - results land in four [128, ~2600] quarter tiles; each quarter DMAs
    to HBM as soon as its last column is written (the 21.3 MB/core
    output write is the roofline term).
"""

import os
import sys

import numpy as np

for _p in (
    "/root/.axon_site",
    "/root/.axon_site/_ro/trn_rl_repo",
    "/root/.axon_site/_ro/pypackages",
    "/opt/trn_rl_repo",
):
    if os.path.isdir(_p) and _p not in sys.path:
        sys.path.append(_p)

import concourse.bacc as bacc
import concourse.tile as tile
from concourse import mybir
from concourse.bass_utils import run_bass_kernel_spmd
from concourse.masks import make_identity

N_CORES = 8
B, F, D = 4096, 26, 32
NPAIR = 325  # F*(F-1)/2
BLOC = B // N_CORES  # 512 batch rows per core
PB = 128  # batch rows per tile (partition dim)
NT = BLOC // PB  # 4 tiles per core
OUTW = NPAIR * D  # 10400 output columns
DA = D + 1  # field block width in augmented x (32 data + 1 one)
NQ = 4  # output quarter tiles per batch tile
# quarter boundaries, 32-aligned (pair-aligned): widths 2624/2592/2592/2592
Q_BOUNDS = [0, 2624, 5216, 7808, OUTW]

FP32 = mybir.dt.float32
FP32R = mybir.dt.float32r
BF16 = mybir.dt.bfloat16

# matmul input dtype: "f32r" (~2 cyc/col, ~1e-3 elemwise err),
# "bf16" (1 cyc/col, ~1e-2), "f32" (exact, 4 cyc/col)
MM_MODE = os.environ.get("BILIN_MM_MODE", "f32r")
_MM_DT = {"bf16": BF16, "f32r": FP32R, "f32": FP32}[MM_MODE]

# 3-field stacks for block-diagonal matmuls (field 24 alone)
GROUPS = [tuple(range(g, min(g + 3, 25))) for g in range(0, 25, 3)]
# field-aligned wbs column chunks (zero-fill + scatter pipelining)
WB_CHUNKS = [(0, 3), (3, 7), (7, 13), (13, 25)]
# (group, chunk_idx) pairs whose psum*v_j multiply runs on GpSimd
OFFLOAD = {(0, 0), (0, 1), (0, 2), (0, 3), (0, 4), (1, 0), (1, 1)}


def _pstart(f: int) -> int:
    # first pair index whose i == f (pairs sorted by (i, j))
    return 25 * f - f * (f - 1) // 2


def _nf(f: int) -> int:
    return (25 - f) * D  # output columns owned by field f


def _chunks(width: int):
    """Split width into psum chunks, each <=512, >=256 where possible, %32==0."""
    out = []
    rem = width
    while rem > 0:
        if rem <= 512:
            c = rem
        elif rem >= 768:
            c = 512
        else:
            c = rem - 256
        out.append(c)
        rem -= c
    starts = []
    s = 0
    for c in out:
        starts.append((s, s + c))
        s += c
    return starts


def pack_weights(W: np.ndarray, b: np.ndarray) -> np.ndarray:
    """Build compact wbs[33, OUTW]: rows 0..31 = W[p].T per pair, row 32 = bias."""
    W = np.ascontiguousarray(W, dtype=np.float32)
    b = np.ascontiguousarray(b, dtype=np.float32)
    wbs = np.empty((33, OUTW), dtype=np.float32)
    wbs[:D] = W.transpose(2, 0, 1).reshape(D, OUTW)
    wbs[D] = b.reshape(OUTW)
    if MM_MODE == "bf16":
        import ml_dtypes

        wbs = wbs.astype(ml_dtypes.bfloat16)
    return wbs


def _emit(tc: tile.TileContext, out_ap, xs_ap, wbs_ap):
    from contextlib import ExitStack

    nc = tc.nc
    mm_dt = _MM_DT

    with ExitStack() as ctx:
        const = ctx.enter_context(tc.tile_pool(name="const", bufs=1))
        xp = ctx.enter_context(tc.tile_pool(name="xp", bufs=4))
        stg = ctx.enter_context(tc.tile_pool(name="stg", bufs=2))
        op = ctx.enter_context(tc.tile_pool(name="op", bufs=2))
        scr = ctx.enter_context(tc.tile_pool(name="scr", bufs=4))
        tps = ctx.enter_context(tc.tile_pool(name="tps", bufs=2, space="PSUM"))
        mps = ctx.enter_context(tc.tile_pool(name="mps", bufs=6, space="PSUM"))

        if mm_dt == FP32R:
            # gpsimd memset can't write f32r; build fp32 then round via ACT
            ident_nat = const.tile([128, 128], FP32, tag="idnat", name="ident_nat")
            make_identity(nc, ident_nat)
            ident = const.tile([128, 128], mm_dt, tag="ident", name="ident")
            nc.scalar.copy(ident, ident_nat)
            ones_nat = const.tile([128, F], FP32, tag="ones", name="ones_nat")
            nc.gpsimd.memset(ones_nat, 1.0)
        else:
            ident = const.tile([128, 128], mm_dt, tag="ident", name="ident")
            make_identity(nc, ident)
            ones_nat = None

        # x loads go first so they are not queued behind the weight scatters
        def load_x(bt):
            b0 = bt * PB
            # x loaded contiguously (clean 3.3KB/partition DMA runs)
            x_nat = xp.tile([PB, F, D], FP32, tag="xnat", name=f"xnat{bt}")
            nc.sync.dma_start(out=x_nat, in_=xs_ap[b0 : b0 + PB])
            return x_nat

        x_nats = [load_x(bt) for bt in range(NT)]

        def prep_x(bt, x_nat):
            # cast copy into mm dtype with a ones column per field block
            x_mm = xp.tile([PB, F, DA], mm_dt, tag="xmm", name=f"xmm{bt}")
            nc.scalar.copy(x_mm[:, :, 0:D], x_nat)
            if mm_dt == FP32R:
                nc.scalar.copy(x_mm[:, :, D], ones_nat)
            else:
                nc.gpsimd.memset(x_mm[:, :, D : D + 1], 1.0)
            return x_mm

        def transposes(bt, x_mm):
            # transpose each field stack -> [33*len(fs), 128] staging (PE+ACT)
            stgs = []
            for gi, fs in enumerate(GROUPS):
                kg = 33 * len(fs)
                tin = x_mm[:, fs[0] : fs[0] + len(fs), :].rearrange("p a b -> p (a b)")
                ps_t = tps.tile([kg, PB], mm_dt, tag="tp", name=f"pst{bt}_{gi}")
                nc.tensor.transpose(ps_t, tin, ident)
                stg_g = stg.tile([kg, PB], mm_dt, tag=f"sg{gi}", name=f"stg{bt}_{gi}")
                nc.scalar.copy(stg_g, ps_t)
                stgs.append(stg_g)
            return stgs

        x_mms = {0: prep_x(0, x_nats[0])}
        stgs_by_bt = {0: transposes(0, x_mms[0])}

        # block-diagonal weight image: zero-fill, then scatter the compact
        # HBM wbs into phase rows 33*(f%3) per field, chunk by chunk.
        # (gpsimd/DVE memsets can't write f32r -> zero an fp32 scratch and
        # round it in via ACT (chunk 0, needed first) / DVE copies.)
        wbs3 = const.tile([99, OUTW], mm_dt, tag="wbs3", name="wbs3")
        zeng = [nc.gpsimd, nc.vector, nc.gpsimd, nc.vector]
        for i, (f0, f1) in enumerate(WB_CHUNKS):
            c0 = _pstart(f0) * D
            c1 = (_pstart(f1) if f1 < 25 else NPAIR) * D
            if mm_dt == FP32R:
                zc = scr.tile(
                    [99, c1 - c0], FP32, tag=f"zc{i % 2}", bufs=1, name=f"zc{f0}"
                )
                zeng[i].memset(zc, 0.0)
                if i == 0:
                    nc.scalar.copy(wbs3[:, c0:c1], zc)
                else:
                    nc.vector.tensor_copy(wbs3[:, c0:c1], zc)
            else:
                zeng[i].memset(wbs3[:, c0:c1], 0.0)
        for f in range(25):
            ph = f % 3
            base = _pstart(f) * D
            nc.sync.dma_start(
                out=wbs3[33 * ph : 33 * ph + 33, base : base + _nf(f)],
                in_=wbs_ap[:, base : base + _nf(f)],
            )

        for bt in range(NT):
            b0 = bt * PB
            x_nat = x_nats[bt]
            stgs = stgs_by_bt.pop(bt)

            quarters = [
                op.tile(
                    [PB, Q_BOUNDS[q + 1] - Q_BOUNDS[q]],
                    FP32,
                    tag=f"osb{q}",
                    name=f"osb{bt}_{q}",
                )
                for q in range(NQ)
            ]

            def tt(col0, col1, src, scol0, j0, eng):
                """quarters[col0:col1) = src * v_j, split at quarter edges."""
                c = col0
                while c < col1:
                    q = next(i for i in range(NQ) if c < Q_BOUNDS[i + 1])
                    e = min(col1, Q_BOUNDS[q + 1])
                    nj = (e - c) // D
                    j = j0 + (c - col0) // D
                    eng.tensor_mul(
                        quarters[q][:, c - Q_BOUNDS[q] : e - Q_BOUNDS[q]],
                        src[:, scol0 + (c - col0) : scol0 + (e - col0)],
                        x_nat[:, j : j + nj, :],
                    )
                    c = e

            for gi, fs in enumerate(GROUPS):
                if gi == 5 and bt + 1 < NT:
                    # interleave next tile's transposes to keep the PE dense
                    x_mms[bt + 1] = prep_x(bt + 1, x_nats[bt + 1])
                    stgs_by_bt[bt + 1] = transposes(bt + 1, x_mms[bt + 1])
                kg = 33 * len(fs)
                gbase = _pstart(fs[0]) * D
                width = sum(_nf(f) for f in fs)
                offs = []
                o = 0
                for f in fs:
                    offs.append(o)
                    o += _nf(f)
                for ci, (c0, c1) in enumerate(_chunks(width)):
                    offload = (gi, ci) in OFFLOAD
                    ps_m = mps.tile(
                        [PB, c1 - c0], FP32, tag="mp", name=f"psm{bt}_{gi}_{c0}"
                    )
                    nc.tensor.matmul(
                        ps_m,
                        stgs[gi],
                        wbs3[:kg, gbase + c0 : gbase + c1],
                        start=True,
                        stop=True,
                    )
                    if offload:
                        # GpSimd cannot read PSUM: stage via ScalarE copy
                        sc = scr.tile(
                            [PB, c1 - c0], FP32, tag="sc", name=f"sc{bt}_{gi}_{c0}"
                        )
                        nc.scalar.copy(sc, ps_m)
                        src = sc
                    else:
                        src = ps_m
                    eng = nc.gpsimd if offload else nc.vector
                    for f, off in zip(fs, offs):
                        s0 = max(c0, off)
                        s1 = min(c1, off + _nf(f))
                        if s0 >= s1:
                            continue
                        tt(
                            gbase + s0,
                            gbase + s1,
                            src,
                            s0 - c0,
                            f + 1 + (s0 - off) // D,
                            eng,
                        )

            for q in range(NQ):
                qw = Q_BOUNDS[q + 1] - Q_BOUNDS[q]
                if bt == NT - 1:
                    # last tile: two half-DMAs per quarter to shrink the tail
                    h = (qw // 2) // D * D
                    for h0, h1 in ((0, h), (h, qw)):
                        nc.sync.dma_start(
                            out=out_ap[
                                b0 : b0 + PB, Q_BOUNDS[q] + h0 : Q_BOUNDS[q] + h1
                            ],
                            in_=quarters[q][:, h0:h1],
                        )
                else:
                    nc.sync.dma_start(
                        out=out_ap[b0 : b0 + PB, Q_BOUNDS[q] : Q_BOUNDS[q + 1]],
                        in_=quarters[q],
                    )


_CACHE = {}


def _build():
    if "nc" in _CACHE:
        return _CACHE["nc"]
    nc = bacc.Bacc("TRN2", target_bir_lowering=False, debug=False)
    xs = nc.dram_tensor("xs", [BLOC, F, D], FP32, kind="ExternalInput").ap()
    wbs = nc.dram_tensor("wbs", [33, OUTW], _MM_DT, kind="ExternalInput").ap()
    out = nc.dram_tensor("out", [BLOC, OUTW], FP32, kind="ExternalOutput").ap()
    with tile.TileContext(nc) as tc:
        _emit(tc, out, xs, wbs)
    nc.compile()
    _CACHE["nc"] = nc
    return nc


def run(
    x: np.ndarray,
    W: np.ndarray,
    b: np.ndarray,
    trace: bool = False,
    tmpdir: str | None = None,
):
    """Shard, execute on 8 cores, gather. Returns (out, results_obj)."""
    x = np.ascontiguousarray(x, dtype=np.float32)
    wbs = pack_weights(W, b)
    nc = _build()
    in_maps = [
        {"xs": x[c * BLOC : (c + 1) * BLOC], "wbs": wbs} for c in range(N_CORES)
    ]
    res = run_bass_kernel_spmd(
        nc, in_maps, core_ids=list(range(N_CORES)), trace=trace, tmpdir=tmpdir
    )
    parts = [res.results[c]["out"].reshape(BLOC, NPAIR, D) for c in range(N_CORES)]
    out = np.concatenate(parts, axis=0).astype(np.float32, copy=False)
    return out, res


def kernel(x: np.ndarray, W: np.ndarray, b: np.ndarray) -> np.ndarray:
    out, _ = run(x, W, b, trace=False)
    return out


if __name__ == "__main__":
    rng = np.random.default_rng(0)
    x = rng.standard_normal((B, F, D), dtype=np.float32)
    W = rng.standard_normal((NPAIR, D, D), dtype=np.float32) / np.sqrt(D)
    b = rng.standard_normal((NPAIR, D), dtype=np.float32) * 0.01
    out = kernel(x, W, b)
    print("out", out.shape, out.dtype)


# revision 17
# speedup vs baseline: 1.1974x; 1.0005x over previous
"""Bass/Trainium2 kernel for nn_BilinearInteraction.

Computes out[b, p, :] = (x[b, i_p, :] @ W[p].T + bias[p]) * x[b, j_p, :]
for the 325 upper-triangular field pairs (i_p < j_p), batch B=4096,
F=26 fields, D=32 embed dim.

Strategy (data parallel over 8 NeuronCores, 512 batch rows each):
  - batch rows live on SBUF partitions (4 tiles of 128 rows per core).
  - pairs sharing the same i-field are contiguous in p, so for each field
    f the output columns [pstart(f)*32, (pstart(f)+25-f)*32) are produced
    by ONE stationary operand: xT_f = transpose(x[:, f, :]) augmented
    with a row of ones (bias trick -> K=33 contraction).
  - fields are stacked 3-at-a-time (block-diagonal, K=99) so one PE
    transpose + one psum->sbuf copy serves 3 fields. The block-diagonal
    weight image lives only in SBUF: HBM holds the compact wbs[33,10400]
    (rows 0..31 = W[p].T per pair, row 32 = bias); 25 small DMAs scatter
    it into the zeroed [99,10400] SBUF tile at phase offsets 33*(f%3).
  - matmul inputs are float32r (single-pass fp32 on the PE, ~2 cyc/col).
  - psum * v_j (v_j = natural column slices of x) runs on VectorE, with
    the first two field-groups offloaded to GpSimd (via a ScalarE
    psum->sbuf copy) to keep VectorE off the critical path.
  - results land in four [128, ~2600] quarter tiles; each quarter DMAs
    to HBM as soon as its last column is written (the 21.3 MB/core
    output write is the roofline term).
"""

import os
import sys

import numpy as np

for _p in (
    "/root/.axon_site",
    "/root/.axon_site/_ro/trn_rl_repo",
    "/root/.axon_site/_ro/pypackages",
    "/opt/trn_rl_repo",
):
    if os.path.isdir(_p) and _p not in sys.path:
        sys.path.append(_p)

import concourse.bacc as bacc
import concourse.tile as tile
from concourse import mybir
from concourse.bass_utils import run_bass_kernel_spmd
from concourse.masks import make_identity

N_CORES = 8
B, F, D = 4096, 26, 32
NPAIR = 325  # F*(F-1)/2
BLOC = B // N_CORES  # 512 batch rows per core
PB = 128  # batch rows per tile (partition dim)
NT = BLOC // PB  # 4 tiles per core
OUTW = NPAIR * D  # 10400 output columns
DA = D + 1  # field block width in augmented x (32 data + 1 one)
NQ = 4  # output quarter tiles per batch tile
# quarter boundaries, 32-aligned (pair-aligned): widths 2624/2592/2592/2592
Q_BOUNDS = [0, 2624, 5216, 7808, OUTW]

FP32 = mybir.dt.float32
FP32R = mybir.dt.float32r
BF16 = mybir.dt.bfloat16

# matmul input dtype: "f32r" (~2 cyc/col, ~1e-3 elemwise err),
# "bf16" (1 cyc/col, ~1e-2), "f32" (exact, 4 cyc/col)
MM_MODE = os.environ.get("BILIN_MM_MODE", "f32r")
_MM_DT = {"bf16": BF16, "f32r": FP32R, "f32": FP32}[MM_MODE]

# 3-field stacks for block-diagonal matmuls (field 24 alone)
GROUPS = [tuple(range(g, min(g + 3, 25))) for g in range(0, 25, 3)]
# field-aligned wbs column chunks (zero-fill + scatter pipelining)
WB_CHUNKS = [(0, 3), (3, 7), (7, 13), (13, 25)]
# (group, chunk_idx) pairs whose psum*v_j multiply runs on GpSimd
OFFLOAD = {(0, 0), (0, 1), (0, 2), (0, 3), (0, 4), (1, 0), (1, 1)}


def _pstart(f: int) -> int:
    # first pair index whose i == f (pairs sorted by (i, j))
    return 25 * f - f * (f - 1) // 2


def _nf(f: int) -> int:
    return (25 - f) * D  # output columns owned by field f


def _chunks(width: int):
    """Split width into psum chunks, each <=512, >=256 where possible, %32==0."""
    out = []
    rem = width
    while rem > 0:
        if rem <= 512:
            c = rem
        elif rem >= 768:
            c = 512
        else:
            c = rem - 256
        out.append(c)
        rem -= c
    starts = []
    s = 0
    for c in out:
        starts.append((s, s + c))
        s += c
    return starts


def pack_weights(W: np.ndarray, b: np.ndarray) -> np.ndarray:
    """Build block-diagonal wbs3[99, OUTW]: field f's [33, nf] block (rows
    0..31 = W[p].T per pair, row 32 = bias) at phase rows 33*(f%3)."""
    W = np.ascontiguousarray(W, dtype=np.float32)
    b = np.ascontiguousarray(b, dtype=np.float32)
    wbs = np.zeros((99, OUTW), dtype=np.float32)
    for f in range(25):
        ph = f % 3
        p0 = _pstart(f)
        npair = 25 - f
        base = p0 * D
        blk = W[p0 : p0 + npair].transpose(2, 0, 1).reshape(D, npair * D)
        wbs[33 * ph : 33 * ph + D, base : base + npair * D] = blk
        wbs[33 * ph + D, base : base + npair * D] = b[p0 : p0 + npair].reshape(-1)
    if MM_MODE == "bf16":
        import ml_dtypes

        wbs = wbs.astype(ml_dtypes.bfloat16)
    return wbs


def _emit(tc: tile.TileContext, out_ap, xs_ap, wbs_ap):
    from contextlib import ExitStack

    nc = tc.nc
    mm_dt = _MM_DT

    with ExitStack() as ctx:
        const = ctx.enter_context(tc.tile_pool(name="const", bufs=1))
        xp = ctx.enter_context(tc.tile_pool(name="xp", bufs=4))
        stg = ctx.enter_context(tc.tile_pool(name="stg", bufs=2))
        op = ctx.enter_context(tc.tile_pool(name="op", bufs=2))
        scr = ctx.enter_context(tc.tile_pool(name="scr", bufs=4))
        tps = ctx.enter_context(tc.tile_pool(name="tps", bufs=2, space="PSUM"))
        mps = ctx.enter_context(tc.tile_pool(name="mps", bufs=6, space="PSUM"))

        if mm_dt == FP32R:
            # gpsimd memset can't write f32r; build fp32 then round via ACT
            ident_nat = const.tile([128, 128], FP32, tag="idnat", name="ident_nat")
            make_identity(nc, ident_nat)
            ident = const.tile([128, 128], mm_dt, tag="ident", name="ident")
            nc.scalar.copy(ident, ident_nat)
            ones_nat = const.tile([128, F], FP32, tag="ones", name="ones_nat")
            nc.gpsimd.memset(ones_nat, 1.0)
        else:
            ident = const.tile([128, 128], mm_dt, tag="ident", name="ident")
            make_identity(nc, ident)
            ones_nat = None

        # x loads go first so they are not queued behind the weight scatters
        def load_x(bt):
            b0 = bt * PB
            # x loaded contiguously (clean 3.3KB/partition DMA runs)
            x_nat = xp.tile([PB, F, D], FP32, tag="xnat", name=f"xnat{bt}")
            nc.sync.dma_start(out=x_nat, in_=xs_ap[b0 : b0 + PB])
            return x_nat

        x_nats = [load_x(bt) for bt in range(NT)]

        def prep_x(bt, x_nat):
            # cast copy into mm dtype with a ones column per field block
            x_mm = xp.tile([PB, F, DA], mm_dt, tag="xmm", name=f"xmm{bt}")
            nc.scalar.copy(x_mm[:, :, 0:D], x_nat)
            if mm_dt == FP32R:
                nc.scalar.copy(x_mm[:, :, D], ones_nat)
            else:
                nc.gpsimd.memset(x_mm[:, :, D : D + 1], 1.0)
            return x_mm

        def transposes(bt, x_mm):
            # transpose each field stack -> [33*len(fs), 128] staging (PE+ACT)
            stgs = []
            for gi, fs in enumerate(GROUPS):
                kg = 33 * len(fs)
                tin = x_mm[:, fs[0] : fs[0] + len(fs), :].rearrange("p a b -> p (a b)")
                ps_t = tps.tile([kg, PB], mm_dt, tag="tp", name=f"pst{bt}_{gi}")
                nc.tensor.transpose(ps_t, tin, ident)
                stg_g = stg.tile([kg, PB], mm_dt, tag=f"sg{gi}", name=f"stg{bt}_{gi}")
                nc.scalar.copy(stg_g, ps_t)
                stgs.append(stg_g)
            return stgs

        x_mms = {0: prep_x(0, x_nats[0])}
        stgs_by_bt = {0: transposes(0, x_mms[0])}

        # block-diagonal weight image, host-packed; 4 big column-chunk DMAs
        # so the first groups' matmuls unblock after ~1 MB, not the full 4 MB
        wbs3 = const.tile([99, OUTW], mm_dt, tag="wbs3", name="wbs3")
        for f0, f1 in WB_CHUNKS:
            c0 = _pstart(f0) * D
            c1 = (_pstart(f1) if f1 < 25 else NPAIR) * D
            nc.sync.dma_start(out=wbs3[:, c0:c1], in_=wbs_ap[:, c0:c1])

        for bt in range(NT):
            b0 = bt * PB
            x_nat = x_nats[bt]
            stgs = stgs_by_bt.pop(bt)

            quarters = [
                op.tile(
                    [PB, Q_BOUNDS[q + 1] - Q_BOUNDS[q]],
                    FP32,
                    tag=f"osb{q}",
                    name=f"osb{bt}_{q}",
                )
                for q in range(NQ)
            ]

            def tt(col0, col1, src, scol0, j0, eng):
                """quarters[col0:col1) = src * v_j, split at quarter edges."""
                c = col0
                while c < col1:
                    q = next(i for i in range(NQ) if c < Q_BOUNDS[i + 1])
                    e = min(col1, Q_BOUNDS[q + 1])
                    nj = (e - c) // D
                    j = j0 + (c - col0) // D
                    eng.tensor_mul(
                        quarters[q][:, c - Q_BOUNDS[q] : e - Q_BOUNDS[q]],
                        src[:, scol0 + (c - col0) : scol0 + (e - col0)],
                        x_nat[:, j : j + nj, :],
                    )
                    c = e

            for gi, fs in enumerate(GROUPS):
                if gi == 5 and bt + 1 < NT:
                    # interleave next tile's transposes to keep the PE dense
                    x_mms[bt + 1] = prep_x(bt + 1, x_nats[bt + 1])
                    stgs_by_bt[bt + 1] = transposes(bt + 1, x_mms[bt + 1])
                kg = 33 * len(fs)
                gbase = _pstart(fs[0]) * D
                width = sum(_nf(f) for f in fs)
                offs = []
                o = 0
                for f in fs:
                    offs.append(o)
                    o += _nf(f)
                for ci, (c0, c1) in enumerate(_chunks(width)):
                    offload = (gi, ci) in OFFLOAD
                    ps_m = mps.tile(
                        [PB, c1 - c0], FP32, tag="mp", name=f"psm{bt}_{gi}_{c0}"
                    )
                    nc.tensor.matmul(
                        ps_m,
                        stgs[gi],
                        wbs3[:kg, gbase + c0 : gbase + c1],
                        start=True,
                        stop=True,
                    )
                    if offload:
                        # GpSimd cannot read PSUM: stage via ScalarE copy
                        sc = scr.tile(
                            [PB, c1 - c0], FP32, tag="sc", name=f"sc{bt}_{gi}_{c0}"
                        )
                        nc.scalar.copy(sc, ps_m)
                        src = sc
                    else:
                        src = ps_m
                    eng = nc.gpsimd if offload else nc.vector
                    for f, off in zip(fs, offs):
                        s0 = max(c0, off)
                        s1 = min(c1, off + _nf(f))
                        if s0 >= s1:
                            continue
                        tt(
                            gbase + s0,
                            gbase + s1,
                            src,
                            s0 - c0,
                            f + 1 + (s0 - off) // D,
                            eng,
                        )

            for q in range(NQ):
                qw = Q_BOUNDS[q + 1] - Q_BOUNDS[q]
                if bt == NT - 1:
                    # last tile: two half-DMAs per quarter to shrink the tail
                    h = (qw // 2) // D * D
                    for h0, h1 in ((0, h), (h, qw)):
                        nc.sync.dma_start(
                            out=out_ap[
                                b0 : b0 + PB, Q_BOUNDS[q] + h0 : Q_BOUNDS[q] + h1
                            ],
                            in_=quarters[q][:, h0:h1],
                        )
                else:
                    nc.sync.dma_start(
                        out=out_ap[b0 : b0 + PB, Q_BOUNDS[q] : Q_BOUNDS[q + 1]],
                        in_=quarters[q],
                    )


_CACHE = {}


def _build():
    if "nc" in _CACHE:
        return _CACHE["nc"]
    nc = bacc.Bacc("TRN2", target_bir_lowering=False, debug=False)
    xs = nc.dram_tensor("xs", [BLOC, F, D], FP32, kind="ExternalInput").ap()
    wbs = nc.dram_tensor("wbs", [99, OUTW], _MM_DT, kind="ExternalInput").ap()
    out = nc.dram_tensor("out", [BLOC, OUTW], FP32, kind="ExternalOutput").ap()
    with tile.TileContext(nc) as tc:
        _emit(tc, out, xs, wbs)
    nc.compile()
    _CACHE["nc"] = nc
    return nc


def run(
    x: np.ndarray,
    W: np.ndarray,
    b: np.ndarray,
    trace: bool = False,
    tmpdir: str | None = None,
):
    """Shard, execute on 8 cores, gather. Returns (out, results_obj)."""
    x = np.ascontiguousarray(x, dtype=np.float32)
    wbs = pack_weights(W, b)
    nc = _build()
    in_maps = [
        {"xs": x[c * BLOC : (c + 1) * BLOC], "wbs": wbs} for c in range(N_CORES)
    ]
    res = run_bass_kernel_spmd(
        nc, in_maps, core_ids=list(range(N_CORES)), trace=trace, tmpdir=tmpdir
    )
    parts = [res.results[c]["out"].reshape(BLOC, NPAIR, D) for c in range(N_CORES)]
    out = np.concatenate(parts, axis=0).astype(np.float32, copy=False)
    return out, res


def kernel(x: np.ndarray, W: np.ndarray, b: np.ndarray) -> np.ndarray:
    out, _ = run(x, W, b, trace=False)
    return out


if __name__ == "__main__":
    rng = np.random.default_rng(0)
    x = rng.standard_normal((B, F, D), dtype=np.float32)
    W = rng.standard_normal((NPAIR, D, D), dtype=np.float32) / np.sqrt(D)
    b = rng.standard_normal((NPAIR, D), dtype=np.float32) * 0.01
    out = kernel(x, W, b)
    print("out", out.shape, out.dtype)
